# revision 23
# baseline (speedup 1.0000x reference)
"""FaceConvDemodulated — Trainium2 Bass kernel (8 NeuronCores, SPMD).

out[n, o] = sum_{k,i} padded[nbr[n,k], i] * w_demod[o, i, k] + bias[o]
  where w_demod = weight * rsqrt(sum_{i,k} weight^2 + 1e-8)  (per output ch.)

Sharding: data-parallel over faces — 6272 faces/core (50176 padded), the
padded fp16 feature table + weights replicated on every core.

Device pipeline per core:
- Gather: SWDGE dma_gather (row mode, no xbar) fetches each (face, tap)
  row of the table by index. Indices are int16 in this ucode, so the
  50001-row table is split: table A = rows 0..32766 + a zero row, table
  B = a zero row + rows 32767..50000; idxA = min(idx, 32767), idxB =
  max(idx - 32766, 0). Exactly one of the two gathered rows is nonzero,
  so A + B reconstructs the row exactly. A-gathers run on SWDGE queue 2,
  B-gathers on queue 3 (concurrent row gathers on separate queues are
  race-free, unlike transpose-mode gathers which share the xbar).
  Descriptor generation on the GPSIMD engine is the kernel's critical
  path (~5 ns/index, engine-serial).
- Transpose: PE transpose (matmul-by-identity) flips each gathered
  [face, i] 128x128 tile to [i, face] in PSUM; A and B are transposed
  separately so neither waits on the other's gather. ScalarE copies the
  A-transpose PSUM->SBUF, VectorE adds the B-transpose into it (the
  exact A+B merge), yielding the fp16 lhsT tile.
- Matmul: psum[f=128, o=256] accumulates 18 fp16 matmuls (9 taps x 2
  feature chunks): lhsT = gathered-T chunk [i=128, f=128], rhs = wT tile
  [i=128, o=256]. Bias is added during the PSUM->SBUF copy; the output
  leaves in natural [face, o] fp32 layout.
- Weight demodulation on device: sum-of-squares via a ones-matmul (the
  128x128 ones lhsT broadcasts the per-o sums to all partitions), then
  Sqrt(+1e-8) on ScalarE, reciprocal on VectorE, scale + fp16 cast.

Host side does only layout/dtype prep: building the padded table (the
reference's cumsum rank mapping), fp16 cast, the A/B table split, index
clamping, and the 16-wrapped x8-replicated index tiles the gather ucode
expects; plus the final shard concatenation.
"""

import numpy as np

N_FACES = 50000
C = 256
K = 9
PAD_SIZE = N_FACES + 1
N_CORES = 8
SHARD = 6272            # 49 * 128 faces per core; 8 * 6272 = 50176
N_PAD_TOTAL = N_CORES * SHARD
SPLIT = 32767           # table A holds rows 0..SPLIT-1 plus a zero row
B_ROWS = PAD_SIZE - SPLIT + 1   # zero row + rows SPLIT..50000 -> 17235
SUPER = 512             # faces per gather batch (SWDGE ring limit ~6144 idxs)
SUPERS = [SUPER] * (SHARD // SUPER) + ([SHARD % SUPER] if SHARD % SUPER else [])

_compiled = None


def _build():
    import concourse.mybir as mybir
    import concourse.tile as tile
    from concourse import bacc
    from concourse.masks import make_identity

    f32, f16 = mybir.dt.float32, mybir.dt.float16
    i16 = mybir.dt.int16

    nc = bacc.Bacc("TRN2", target_bir_lowering=False, debug=False,
                   num_devices=N_CORES, num_swdge_queues=4)

    IDXCOLS = SHARD * K // 16   # 3528
    d_tableA = nc.dram_tensor("tableA", [SPLIT + 1, C], f16,
                              kind="ExternalInput")
    d_tableB = nc.dram_tensor("tableB", [B_ROWS, C], f16,
                              kind="ExternalInput")
    d_wT = nc.dram_tensor("wT", [2 * K * 128, C], f32, kind="ExternalInput")
    d_bias = nc.dram_tensor("bias", [128, C], f32, kind="ExternalInput")
    d_idxA = nc.dram_tensor("idxA", [128, IDXCOLS], i16, kind="ExternalInput")
    d_idxB = nc.dram_tensor("idxB", [128, IDXCOLS], i16, kind="ExternalInput")
    d_out = nc.dram_tensor("out", [SHARD, C], f32, kind="ExternalOutput")

    NT = 2 * K  # 18 weight tiles of [128, C]

    with tile.TileContext(nc) as tc:
        with (
            tc.tile_pool(name="const", bufs=1) as cpool,
            tc.tile_pool(name="wstream", bufs=3) as wpool,
            tc.tile_pool(name="gather", bufs=3) as gpool,
            tc.tile_pool(name="gtp", bufs=6) as gtpool,
            tc.tile_pool(name="outp", bufs=4) as opool,
            tc.tile_pool(name="psum", bufs=3, space="PSUM") as pspool,
            tc.tile_pool(name="psumt", bufs=2, space="PSUM") as ptpool,
            tc.tile_pool(name="psumtb", bufs=2, space="PSUM") as ptpoolB,
            tc.tile_pool(name="wpsum", bufs=1, space="PSUM") as wps,
        ):
            # ---- constants / inputs ----
            idxA = cpool.tile([128, IDXCOLS], i16)
            idxB = cpool.tile([128, IDXCOLS], i16)
            bias_sb = cpool.tile([128, C], f32)
            nc.sync.dma_start(out=idxA[:], in_=d_idxA[:])
            nc.sync.dma_start(out=idxB[:], in_=d_idxB[:])
            nc.sync.dma_start(out=bias_sb[:], in_=d_bias[:])

            # ---- weight demodulation ----
            ones = cpool.tile([128, 128], f32)
            nc.vector.memset(ones[:], 1.0)
            sq_ps = wps.tile([128, C], f32, space="PSUM")
            w32s = []
            for t in range(NT):
                w32 = wpool.tile([128, C], f32, tag="w32", bufs=NT)
                nc.sync.dma_start(out=w32[:],
                                  in_=d_wT[t * 128:(t + 1) * 128, :])
                sq = wpool.tile([128, C], f32, tag="sq")
                nc.vector.tensor_mul(out=sq[:], in0=w32[:], in1=w32[:])
                # psum[m, o] = sum_c sq[c, o] for every m (row-broadcast sum)
                nc.tensor.matmul(out=sq_ps[:], lhsT=ones[:], rhs=sq[:],
                                 start=(t == 0), stop=(t == NT - 1))
                w32s.append(w32)
            denom = cpool.tile([128, C], f32)
            eps = cpool.tile([128, 1], f32)
            nc.vector.memset(eps[:], 1e-8)
            nc.scalar.activation(denom[:], sq_ps[:],
                                 mybir.ActivationFunctionType.Sqrt,
                                 bias=eps[:])
            dcoef = cpool.tile([128, C], f32)
            nc.vector.reciprocal(dcoef[:], denom[:])
            w16 = cpool.tile([128, NT, C], f16)
            for t in range(NT):
                nc.vector.tensor_mul(out=w16[:, t, :], in0=w32s[t][:],
                                     in1=dcoef[:])

            # identity for PE transposes
            ident = cpool.tile([128, 128], f16)
            make_identity(nc, ident[:])

            # ---- main loop over gather batches ----
            col0 = 0
            row0 = 0
            for sf in SUPERS:
                nI = sf * K
                ncol = nI // 16
                nR = nI // 128          # row-blocks; face-tile ft of tap k
                nF = sf // 128          # lives at row-block k*nF + ft
                bufA = gpool.tile([128, nR, C], f16, tag="bufA")
                bufB = gpool.tile([128, nR, C], f16, tag="bufB")
                nc.gpsimd.dma_gather(
                    out_ap=bufA[:], in_ap=d_tableA[:],
                    idxs_ap=idxA[:, col0:col0 + ncol],
                    num_idxs=nI, num_idxs_reg=nI, elem_size=C,
                    transpose=False, single_packet=False, queue_num=2)
                nc.gpsimd.dma_gather(
                    out_ap=bufB[:], in_ap=d_tableB[:],
                    idxs_ap=idxB[:, col0:col0 + ncol],
                    num_idxs=nI, num_idxs_reg=nI, elem_size=C,
                    transpose=False, single_packet=False, queue_num=3)
                for fb in range(nF):
                    ps = pspool.tile([128, C], f32, space="PSUM")
                    for k in range(K):
                        srcA = bufA[:, k * nF + fb, :]       # [face, i]
                        srcB = bufB[:, k * nF + fb, :]
                        pstA = ptpool.tile([128, C], f16, space="PSUM")
                        pstB = ptpoolB.tile([128, C], f16, space="PSUM")
                        nc.tensor.transpose(pstA[:, 0:128], srcA[:, 0:128],
                                            ident[:])
                        nc.tensor.transpose(pstA[:, 128:256],
                                            srcA[:, 128:256], ident[:])
                        nc.tensor.transpose(pstB[:, 0:128], srcB[:, 0:128],
                                            ident[:])
                        nc.tensor.transpose(pstB[:, 128:256],
                                            srcB[:, 128:256], ident[:])
                        # exact merge: one of the two rows is always zero
                        gt = gtpool.tile([128, C], f16)
                        nc.scalar.activation(
                            gt[:], pstA[:], mybir.ActivationFunctionType.Copy)
                        nc.vector.tensor_add(out=gt[:], in0=gt[:],
                                             in1=pstB[:])
                        nc.tensor.matmul(out=ps[:], lhsT=gt[:, 0:128],
                                         rhs=w16[:, 2 * k, :],
                                         start=(k == 0), stop=False)
                        nc.tensor.matmul(out=ps[:], lhsT=gt[:, 128:256],
                                         rhs=w16[:, 2 * k + 1, :],
                                         start=False, stop=(k == K - 1))
                    ot = opool.tile([128, C], f32)
                    nc.vector.tensor_add(out=ot[:], in0=ps[:], in1=bias_sb[:])
                    nc.sync.dma_start(
                        out=d_out[row0 + fb * 128: row0 + (fb + 1) * 128, :],
                        in_=ot[:])
                col0 += ncol
                row0 += sf

    nc.compile()
    return nc


def _host_prep(x, weight, bias, face_neighborhood, face_is_pad):
    """Pure layout/dtype prep: padded table, split tables, wrapped indices."""
    x = np.asarray(x, np.float32)
    w = np.asarray(weight, np.float32)          # [O, I, 1, K]
    b = np.asarray(bias, np.float32)
    nbr = np.asarray(face_neighborhood).astype(np.int64)   # [N, K]
    pad = np.asarray(face_is_pad).astype(bool)

    # padded feature table, mirroring reference._pad_features
    rank = np.clip(np.cumsum(~pad) - 1, 0, x.shape[0] - 1)
    padded = np.where(pad[:, None], np.float32(0), x[rank]).astype(np.float16)

    tableA = np.zeros((SPLIT + 1, C), np.float16)
    tableA[:SPLIT] = padded[:SPLIT]
    tableB = np.zeros((B_ROWS, C), np.float16)
    tableB[1:] = padded[SPLIT:]

    # transposed weights: row (k*256 + i) -> o
    wT = np.ascontiguousarray(
        np.transpose(w[:, :, 0, :], (2, 1, 0)).reshape(2 * K * 128, C))

    bias_t = np.ascontiguousarray(np.broadcast_to(b[None, :], (128, C)))

    # per-core wrapped index tiles
    nbr_pad = np.full((N_PAD_TOTAL, K), PAD_SIZE - 1, np.int64)
    nbr_pad[:N_FACES] = nbr
    idxA_all, idxB_all = [], []
    for core in range(N_CORES):
        shard = nbr_pad[core * SHARD:(core + 1) * SHARD]      # [SHARD, K]
        cols = []
        r0 = 0
        for sf in SUPERS:
            blk = shard[r0:r0 + sf]                           # [sf, K]
            lst = blk.T.reshape(-1)                           # k-major
            cols.append(lst.reshape(-1, 16))                  # [nI/16, 16]
            r0 += sf
        flat = np.concatenate(cols, axis=0)                   # [IDXCOLS, 16]
        wrapped = flat.T                                      # [16, IDXCOLS]
        a16 = np.minimum(wrapped, SPLIT).astype(np.int16)
        b16 = np.maximum(wrapped - (SPLIT - 1), 0).astype(np.int16)
        idxA_all.append(np.tile(a16, (8, 1)))                 # replicate x8
        idxB_all.append(np.tile(b16, (8, 1)))
    return tableA, tableB, wT, bias_t, idxA_all, idxB_all


def kernel(x, weight, bias, face_neighborhood, face_is_pad, pad_size):
    global _compiled
    from concourse import bass_utils

    if _compiled is None:
        _compiled = _build()
    nc = _compiled

    tableA, tableB, wT, bias_t, idxA_all, idxB_all = _host_prep(
        x, weight, bias, face_neighborhood, face_is_pad)

    in_maps = []
    for core in range(N_CORES):
        in_maps.append({
            "tableA": tableA, "tableB": tableB, "wT": wT, "bias": bias_t,
            "idxA": idxA_all[core], "idxB": idxB_all[core],
        })
    res = bass_utils.run_bass_kernel_spmd(nc, in_maps,
                                          core_ids=list(range(N_CORES)))
    globals()["_last_results"] = res
    out = np.concatenate([r["out"] for r in res.results], axis=0)[:N_FACES]
    return np.ascontiguousarray(out.astype(np.float32))


# revision 24
# speedup vs baseline: 2.1316x; 2.1316x over previous
"""FaceConvDemodulated — Trainium2 Bass kernel (8 NeuronCores, SPMD).

out[n, o] = sum_{k,i} padded[nbr[n,k], i] * w_demod[o, i, k] + bias[o]
  where w_demod = weight * rsqrt(sum_{i,k} weight^2 + 1e-8)  (per output ch.)

Sharding: data-parallel over faces — 6272 faces/core (50176 padded), the
padded fp16 feature table + weights replicated on every core.

Device pipeline per core:
- Gather: SWDGE dma_gather (row mode, no xbar) fetches each (face, tap)
  row of the table by index. Indices are int16 in this ucode, so the
  50001-row table is split: table A = rows 0..32766 + a zero row, table
  B = a zero row + rows 32767..50000; idxA = min(idx, 32767), idxB =
  max(idx - 32766, 0). Exactly one of the two gathered rows is nonzero,
  so A + B reconstructs the row exactly. A-gathers run on SWDGE queue 2,
  B-gathers on queue 3 (concurrent row gathers on separate queues are
  race-free, unlike transpose-mode gathers which share the xbar).
  Descriptor generation on the GPSIMD engine is the kernel's critical
  path (~5 ns/index, engine-serial).
- Transpose: PE transpose (matmul-by-identity) flips each gathered
  [face, i] 128x128 tile to [i, face] in PSUM; A and B are transposed
  separately so neither waits on the other's gather. ScalarE copies the
  A-transpose PSUM->SBUF, VectorE adds the B-transpose into it (the
  exact A+B merge), yielding the fp16 lhsT tile.
- Matmul: psum[f=128, o=256] accumulates 18 fp16 matmuls (9 taps x 2
  feature chunks): lhsT = gathered-T chunk [i=128, f=128], rhs = wT tile
  [i=128, o=256]. Bias is added during the PSUM->SBUF copy; the output
  leaves in natural [face, o] fp32 layout.
- Weight demodulation on device: sum-of-squares via a ones-matmul (the
  128x128 ones lhsT broadcasts the per-o sums to all partitions), then
  Sqrt(+1e-8) on ScalarE, reciprocal on VectorE, scale + fp16 cast.

Host side does only layout/dtype prep: building the padded table (the
reference's cumsum rank mapping), fp16 cast, the A/B table split, index
clamping, and the 16-wrapped x8-replicated index tiles the gather ucode
expects; plus the final shard concatenation.
"""

import numpy as np

N_FACES = 50000
C = 256
K = 9
PAD_SIZE = N_FACES + 1
N_CORES = 8
SHARD = 6272            # 49 * 128 faces per core; 8 * 6272 = 50176
N_PAD_TOTAL = N_CORES * SHARD
ZPAD = 64               # leading all-zero rows per table; dead indices are
                        # position-hashed across them (a single hot zero row
                        # measurably serializes the SDMA gather stream)
SPLIT = 32768 - ZPAD    # table A holds padded rows 0..SPLIT-1 after its zeros
B_ROWS = ZPAD + (PAD_SIZE - SPLIT)   # zeros + rows SPLIT..50000
SUPER = 512             # faces per gather batch (SWDGE ring limit ~6144 idxs)
SUPERS = [SUPER] * (SHARD // SUPER) + ([SHARD % SUPER] if SHARD % SUPER else [])

_compiled = None


def _build():
    import concourse.mybir as mybir
    import concourse.tile as tile
    from concourse import bacc
    from concourse.masks import make_identity

    f32, f16 = mybir.dt.float32, mybir.dt.float16
    i16 = mybir.dt.int16

    nc = bacc.Bacc("TRN2", target_bir_lowering=False, debug=False,
                   num_devices=N_CORES, num_swdge_queues=4)

    IDXCOLS = SHARD * K // 16   # 3528
    d_tableA = nc.dram_tensor("tableA", [ZPAD + SPLIT, C], f16,
                              kind="ExternalInput")
    d_tableB = nc.dram_tensor("tableB", [B_ROWS, C], f16,
                              kind="ExternalInput")
    d_wT = nc.dram_tensor("wT", [2 * K * 128, C], f32, kind="ExternalInput")
    d_bias = nc.dram_tensor("bias", [128, C], f32, kind="ExternalInput")
    d_idxA = nc.dram_tensor("idxA", [128, IDXCOLS], i16, kind="ExternalInput")
    d_idxB = nc.dram_tensor("idxB", [128, IDXCOLS], i16, kind="ExternalInput")
    d_out = nc.dram_tensor("out", [SHARD, C], f32, kind="ExternalOutput")

    NT = 2 * K  # 18 weight tiles of [128, C]

    with tile.TileContext(nc) as tc:
        with (
            tc.tile_pool(name="const", bufs=1) as cpool,
            tc.tile_pool(name="wstream", bufs=3) as wpool,
            tc.tile_pool(name="gather", bufs=3) as gpool,
            tc.tile_pool(name="gtp", bufs=6) as gtpool,
            tc.tile_pool(name="outp", bufs=4) as opool,
            tc.tile_pool(name="psum", bufs=3, space="PSUM") as pspool,
            tc.tile_pool(name="psumt", bufs=2, space="PSUM") as ptpool,
            tc.tile_pool(name="psumtb", bufs=2, space="PSUM") as ptpoolB,
            tc.tile_pool(name="wpsum", bufs=1, space="PSUM") as wps,
        ):
            # ---- constants / inputs ----
            idxA = cpool.tile([128, IDXCOLS], i16)
            idxB = cpool.tile([128, IDXCOLS], i16)
            bias_sb = cpool.tile([128, C], f32)
            nc.sync.dma_start(out=idxA[:], in_=d_idxA[:])
            nc.sync.dma_start(out=idxB[:], in_=d_idxB[:])
            nc.sync.dma_start(out=bias_sb[:], in_=d_bias[:])

            # ---- weight demodulation ----
            ones = cpool.tile([128, 128], f32)
            nc.vector.memset(ones[:], 1.0)
            sq_ps = wps.tile([128, C], f32, space="PSUM")
            w32s = []
            for t in range(NT):
                w32 = wpool.tile([128, C], f32, tag="w32", bufs=NT)
                nc.sync.dma_start(out=w32[:],
                                  in_=d_wT[t * 128:(t + 1) * 128, :])
                sq = wpool.tile([128, C], f32, tag="sq")
                nc.vector.tensor_mul(out=sq[:], in0=w32[:], in1=w32[:])
                # psum[m, o] = sum_c sq[c, o] for every m (row-broadcast sum)
                nc.tensor.matmul(out=sq_ps[:], lhsT=ones[:], rhs=sq[:],
                                 start=(t == 0), stop=(t == NT - 1))
                w32s.append(w32)
            denom = cpool.tile([128, C], f32)
            eps = cpool.tile([128, 1], f32)
            nc.vector.memset(eps[:], 1e-8)
            nc.scalar.activation(denom[:], sq_ps[:],
                                 mybir.ActivationFunctionType.Sqrt,
                                 bias=eps[:])
            dcoef = cpool.tile([128, C], f32)
            nc.vector.reciprocal(dcoef[:], denom[:])
            w16 = cpool.tile([128, NT, C], f16)
            for t in range(NT):
                nc.vector.tensor_mul(out=w16[:, t, :], in0=w32s[t][:],
                                     in1=dcoef[:])

            # identity for PE transposes
            ident = cpool.tile([128, 128], f16)
            make_identity(nc, ident[:])

            # ---- main loop over gather batches ----
            col0 = 0
            row0 = 0
            for sf in SUPERS:
                nI = sf * K
                ncol = nI // 16
                nR = nI // 128          # row-blocks; face-tile ft of tap k
                nF = sf // 128          # lives at row-block k*nF + ft
                bufA = gpool.tile([128, nR, C], f16, tag="bufA")
                bufB = gpool.tile([128, nR, C], f16, tag="bufB")
                nc.gpsimd.dma_gather(
                    out_ap=bufA[:], in_ap=d_tableA[:],
                    idxs_ap=idxA[:, col0:col0 + ncol],
                    num_idxs=nI, num_idxs_reg=nI, elem_size=C,
                    transpose=False, single_packet=False, queue_num=2)
                nc.gpsimd.dma_gather(
                    out_ap=bufB[:], in_ap=d_tableB[:],
                    idxs_ap=idxB[:, col0:col0 + ncol],
                    num_idxs=nI, num_idxs_reg=nI, elem_size=C,
                    transpose=False, single_packet=False, queue_num=3)
                for fb in range(nF):
                    ps = pspool.tile([128, C], f32, space="PSUM")
                    for k in range(K):
                        srcA = bufA[:, k * nF + fb, :]       # [face, i]
                        srcB = bufB[:, k * nF + fb, :]
                        pstA = ptpool.tile([128, C], f16, space="PSUM")
                        pstB = ptpoolB.tile([128, C], f16, space="PSUM")
                        nc.tensor.transpose(pstA[:, 0:128], srcA[:, 0:128],
                                            ident[:])
                        nc.tensor.transpose(pstA[:, 128:256],
                                            srcA[:, 128:256], ident[:])
                        nc.tensor.transpose(pstB[:, 0:128], srcB[:, 0:128],
                                            ident[:])
                        nc.tensor.transpose(pstB[:, 128:256],
                                            srcB[:, 128:256], ident[:])
                        # exact merge: one of the two rows is always zero
                        gt = gtpool.tile([128, C], f16)
                        nc.scalar.activation(
                            gt[:], pstA[:], mybir.ActivationFunctionType.Copy)
                        nc.vector.tensor_add(out=gt[:], in0=gt[:],
                                             in1=pstB[:])
                        nc.tensor.matmul(out=ps[:], lhsT=gt[:, 0:128],
                                         rhs=w16[:, 2 * k, :],
                                         start=(k == 0), stop=False)
                        nc.tensor.matmul(out=ps[:], lhsT=gt[:, 128:256],
                                         rhs=w16[:, 2 * k + 1, :],
                                         start=False, stop=(k == K - 1))
                    ot = opool.tile([128, C], f32)
                    nc.vector.tensor_add(out=ot[:], in0=ps[:], in1=bias_sb[:])
                    nc.sync.dma_start(
                        out=d_out[row0 + fb * 128: row0 + (fb + 1) * 128, :],
                        in_=ot[:])
                col0 += ncol
                row0 += sf

    nc.compile()
    return nc


def _host_prep(x, weight, bias, face_neighborhood, face_is_pad):
    """Pure layout/dtype prep: padded table, split tables, wrapped indices."""
    x = np.asarray(x, np.float32)
    w = np.asarray(weight, np.float32)          # [O, I, 1, K]
    b = np.asarray(bias, np.float32)
    nbr = np.asarray(face_neighborhood).astype(np.int64)   # [N, K]
    pad = np.asarray(face_is_pad).astype(bool)

    # padded feature table, mirroring reference._pad_features
    rank = np.clip(np.cumsum(~pad) - 1, 0, x.shape[0] - 1)
    padded = np.where(pad[:, None], np.float32(0), x[rank]).astype(np.float16)

    tableA = np.zeros((ZPAD + SPLIT, C), np.float16)
    tableA[ZPAD:] = padded[:SPLIT]
    tableB = np.zeros((B_ROWS, C), np.float16)
    tableB[ZPAD:] = padded[SPLIT:]

    # transposed weights: row (k*256 + i) -> o
    wT = np.ascontiguousarray(
        np.transpose(w[:, :, 0, :], (2, 1, 0)).reshape(2 * K * 128, C))

    bias_t = np.ascontiguousarray(np.broadcast_to(b[None, :], (128, C)))

    # per-core wrapped index tiles
    nbr_pad = np.full((N_PAD_TOTAL, K), PAD_SIZE - 1, np.int64)
    nbr_pad[:N_FACES] = nbr
    idxA_all, idxB_all = [], []
    for core in range(N_CORES):
        shard = nbr_pad[core * SHARD:(core + 1) * SHARD]      # [SHARD, K]
        cols = []
        r0 = 0
        for sf in SUPERS:
            blk = shard[r0:r0 + sf]                           # [sf, K]
            lst = blk.T.reshape(-1)                           # k-major
            cols.append(lst.reshape(-1, 16))                  # [nI/16, 16]
            r0 += sf
        flat = np.concatenate(cols, axis=0)                   # [IDXCOLS, 16]
        wrapped = flat.T                                      # [16, IDXCOLS]
        a16f = np.tile(wrapped, (8, 1)).astype(np.int64)      # replicate x8
        spread = ((np.arange(a16f.shape[1])[None, :]
                   + 16 * np.arange(128)[:, None]) % ZPAD)
        a16 = np.where(a16f < SPLIT, a16f + ZPAD, spread).astype(np.int16)
        b16 = np.where(a16f >= SPLIT, a16f - SPLIT + ZPAD,
                       spread).astype(np.int16)
        idxA_all.append(a16)
        idxB_all.append(b16)
    return tableA, tableB, wT, bias_t, idxA_all, idxB_all


def kernel(x, weight, bias, face_neighborhood, face_is_pad, pad_size):
    global _compiled
    from concourse import bass_utils

    if _compiled is None:
        _compiled = _build()
    nc = _compiled

    tableA, tableB, wT, bias_t, idxA_all, idxB_all = _host_prep(
        x, weight, bias, face_neighborhood, face_is_pad)

    in_maps = []
    for core in range(N_CORES):
        in_maps.append({
            "tableA": tableA, "tableB": tableB, "wT": wT, "bias": bias_t,
            "idxA": idxA_all[core], "idxB": idxB_all[core],
        })
    res = bass_utils.run_bass_kernel_spmd(nc, in_maps,
                                          core_ids=list(range(N_CORES)))
    globals()["_last_results"] = res
    out = np.concatenate([r["out"] for r in res.results], axis=0)[:N_FACES]
    return np.ascontiguousarray(out.astype(np.float32))
